# revision 39
# baseline (speedup 1.0000x reference)
"""Trainium2 Bass kernel for per-sample outer-product softmax attention block.

  theta = x @ W_theta + b_theta            [B, 256]
  phi   = x @ W_phi   + b_phi              [B, 256]
  f     = x @ W_f     + b_f                [B, 256]
  scores= softmax(theta[:,:,None]*phi[:,None,:], -1)
  t     = einsum('bij,bj->bi', scores, f)
  out   = x + t @ W_g + b_g                [B, 2048]

Data-parallel over 8 cores (512 samples each, 4 groups of 128).  exp(z) on
|z|<=5.8 is replaced by a degree-7 polynomial (exp(-z/2)-weighted Chebyshev
fit) that factorizes over the rank-1 argument z = theta_i*phi_j:

  num_i = sum_k (a_k theta_i^k) M_k,  M_k = sum_j phi_j^k f_j
  den_i = sum_k (a_k theta_i^k) S_k,  S_k = sum_j phi_j^k
  t_i   = num_i / den_i

Design (v3, CoreSim 30.1us vs 44.6us for the bf16/PE-transpose version):
- All projections are fp8e4m3 DoubleRow matmuls (cost-model 4x over bf16).
  x^T and the weights arrive from the host pre-packed in the DR pair layout
  [p, chunk, h, .] — no on-chip transposes of x.  phi/f come out in
  [j-part, s] layout (for the j-contraction of the moments), theta in
  [s-part, i] (for the per-sample polynomial).
- b_theta and b_g are injected into the PSUM accumulations by 1-partition
  fp8 DR matmuls whose pair dim is a 0-stride AP over a packed bias row
  (single-partition DMAs are pathologically slow, so the zeros row of the
  pair is synthesized); b_phi/b_f ride per-partition on the ACT PSUM->SBUF
  bias-copies.  S_0 = 256 is exact and folded into the polynomial as an
  immediate scalar, killing one chain rung and two moments per group.
- phi-power P-ladder on DVE (needs only phi, starts earliest), Q_k = P_k*f
  per group on Pool; moments via free-size-1 PE matmuls against per-k
  coefficient columns (a_k folded in), evacuated by ACT.
- num/den evaluated in Horner-by-quadratic form: linear pairs
  L_m = c_2m + c_2m+1*theta via 2-scalar tensor_scalar (4x DVE mode,
  ~127ns/op), then a mul/add ladder in u = theta^2 (2x mode).  num ladder
  on DVE, den ladder on Pool; the last group's den runs on DVE, which is
  otherwise drained by then.
- Tail per group: t^T via PE transposes, fp8 DR t @ W_g per 512-chunk with
  b_g DR-injected, into double-buffered 1-bank fin PSUM; ACT evacuates,
  Pool adds the bf16 x residual, output leaves in halves as soon as they
  complete (group 2 via ACT so SP stays clear).  The last group drains
  through a dedicated PSUM pool: DVE adds straight from PSUM and each
  512-chunk streams out on SP immediately; its ttb fp8 cast also runs on
  DVE so it is not queued behind ACT's evacuations.
- DMAs occupy their issuing queue for the whole transfer (+~1.3us pipeline
  latency, +0.9us sem prop), so loads are spread by need-time:
  SP xtp-lo/wfp/bpk/xb0/wgp/xb1-3 + outs, Pool aux/xtp-hi/wtp,
  ACT act-table-burn/wpp.  DVE stays DMA-free.
Engine busy (CoreSim): DVE 21.3us, ACT 20.1, Pool 19.2, SP 18.6, PE 11.9
over a 30.1us span; fp32 only in PSUM, everything else bf16/fp8.
"""

import sys

sys.path.insert(0, "/opt/trn_rl_repo")

import numpy as np
import ml_dtypes

import concourse.bass as bass
import concourse.mybir as mybir
import concourse.tile as tile
from concourse.bass_utils import run_bass_kernel_spmd

F32 = mybir.dt.float32
BF16 = mybir.dt.bfloat16
F8 = mybir.dt.float8e4
NPBF = ml_dtypes.bfloat16
NPF8 = ml_dtypes.float8_e4m3
DR = mybir.MatmulPerfMode.DoubleRow

C = 2048
K = 256
N_CORES = 8
DEG = 7
# monomial coefficients of an exp(-z/2)-weighted Chebyshev fit of exp(z)
# on [-5.8, 5.8] (absolute accuracy where exp is small; the softmax ratio
# forgives relative error where exp is large)
COEFS = [1.2020455598831177, 1.2839308977127075, 0.4332510530948639,
         0.05064962059259415, 0.026252320036292076, 0.015349922701716423,
         0.0030575329437851906, 0.00019841239554807544]
NK = DEG + 1  # 8 coefficient slots
# aux layout: ident[0:128] | coef cols [128:136] | b_phi cols [136:138]
# | b_f cols [138:140]
AUXW = 128 + NK + 4
# bpack (fp8, [1, BPW]): b_theta[0:256] | b_g[256:2304].  The DR pair dim
# is synthesized with a 0-stride AP; the paired ones/zeros operand on the
# other side zeroes the duplicate read.
BPW = 2304


def build_nc(n_samp=512, c_dim=C, split_waits=True):
    nc = bass.Bass()
    with tile.TileContext(nc) as tc:
        _body(tc, nc, n_samp, c_dim)
    if split_waits:
        _split_multi_waits(nc)
    return nc


def _split_multi_waits(nc):
    """walrus embeds at most one sync wait per ISA instruction; move extra
    waits onto preceding same-engine NoOps."""
    for fn in nc.m.functions:
        for blk in fn.blocks:
            new = []
            for ins in blk.instructions:
                si = ins.sync_info
                waits = list(si.on_wait) if si is not None and si.on_wait else []
                if len(waits) > 1:
                    for i, w in enumerate(waits[:-1]):
                        new.append(mybir.InstNoOp(
                            name=f"{ins.name}-w{i}",
                            engine=ins.engine,
                            sync_info=mybir.SyncInfo(on_wait=[w], on_update=[]),
                        ))
                    ins.sync_info = mybir.SyncInfo(
                        on_wait=[waits[-1]], on_update=list(si.on_update or []))
                new.append(ins)
            blk.instructions = new


def _body(tc, nc, n_samp, c_dim):
    from contextlib import ExitStack
    AOP = mybir.AluOpType
    n_grp = n_samp // 128          # 4
    n_ch = c_dim // 256            # 8 DoubleRow chunks

    xtp_d = nc.declare_dram_parameter("xtp", [128, n_ch * 2 * n_samp], F8, isOutput=False)
    xb_d = nc.declare_dram_parameter("xb", [n_samp, c_dim], BF16, isOutput=False)
    wtp_d = nc.declare_dram_parameter("wtp", [128, n_ch * 2 * K], F8, isOutput=False)
    wpp_d = nc.declare_dram_parameter("wpp", [128, n_ch * 2 * K], F8, isOutput=False)
    wfp_d = nc.declare_dram_parameter("wfp", [128, n_ch * 2 * K], F8, isOutput=False)
    wgp_d = nc.declare_dram_parameter("wgp", [128, 2 * c_dim], F8, isOutput=False)
    aux_d = nc.declare_dram_parameter("aux", [128, AUXW], BF16, isOutput=False)
    bpk_d = nc.declare_dram_parameter("bpk", [1, BPW], F8, isOutput=False)
    out_d = nc.declare_dram_parameter("out", [n_samp, c_dim], BF16, isOutput=True)

    ctx = ExitStack()
    with ctx:
        const = ctx.enter_context(tc.tile_pool(name="const", bufs=1))

        xtp_sb = const.tile([128, n_ch, 2, n_samp], F8)
        xb_sb = const.tile([128, n_grp, c_dim], BF16)
        wtp_sb = const.tile([128, n_ch, 2, K], F8)
        wpp_sb = const.tile([128, n_ch, 2, K], F8)
        wfp_sb = const.tile([128, n_ch, 2, K], F8)
        wgp_sb = const.tile([128, 2, c_dim], F8)
        aux_sb = const.tile([128, AUXW], BF16)
        bpk_sb = const.tile([1, BPW], F8)
        ones_col = const.tile([1, 128], BF16)
        nc.vector.memset(ones_col, 1.0)
        onespair = const.tile([1, 2, 128], F8)
        nc.vector.memset(onespair, 0.0)
        nc.vector.memset(onespair[:, 0, :], 1.0)
        ph_w = const.tile([128, 2, n_samp], BF16)
        ff_w = const.tile([128, 2, n_samp], BF16)
        scr = const.tile([1, 128], BF16)

        ident = aux_sb[:, 0:128]

        def bview(lo, n):
            # [1, 2, n] view of bpk with 0-stride pair dim; the paired
            # ones/zeros operand on the other side zeroes the duplicate read
            v = bpk_sb[:, lo:lo + n]
            return bass.AP(tensor=v.tensor, offset=v.offset,
                           ap=[v.ap[0], [0, 2]] + v.ap[1:])
        btp = bview(0, K)
        bgp = lambda cs: bview(256 + cs.start, cs.stop - cs.start)

        # ---- DMA loads.  A DMA occupies its issuing queue for the whole
        # transfer (+~1.3us pipelined latency before, +0.9us sem after);
        # transfers on different queues overlap.  Ordered by need-time:
        #   SP:   xtp chunks 0-3, wfp, bpk, xb0, wgp, xb1-3, out0-2
        #   Pool: aux, xtp chunks 4-7, wtp; then Q-muls/den/adds
        #   ACT:  table burn, wpp; then PSUM->SBUF copies
        xtp_v = xtp_d[:].rearrange("p (c h s) -> p c h s", h=2, s=n_samp)
        hc = n_ch // 2
        nc.sync.dma_start(out=xtp_sb[:, 0:hc], in_=xtp_v[:, 0:hc])
        nc.gpsimd.dma_start(out=aux_sb, in_=aux_d[:])
        nc.gpsimd.dma_start(out=xtp_sb[:, hc:], in_=xtp_v[:, hc:])
        # burn the one-time ACT table load while ACT is otherwise idle
        nc.scalar.activation(scr, ones_col, mybir.ActivationFunctionType.Identity)
        nc.scalar.copy(scr, ones_col)
        nc.scalar.dma_start(out=wpp_sb, in_=wpp_d[:].rearrange(
            "p (c h k) -> p c h k", h=2, k=K))
        nc.sync.dma_start(out=wfp_sb, in_=wfp_d[:].rearrange(
            "p (c h k) -> p c h k", h=2, k=K))
        nc.gpsimd.dma_start(out=wtp_sb, in_=wtp_d[:].rearrange(
            "p (c h k) -> p c h k", h=2, k=K))
        nc.sync.dma_start(out=bpk_sb, in_=bpk_d[:])
        xb_v = xb_d[:].rearrange("(g p) c -> p g c", p=128)
        nc.sync.dma_start(out=xb_sb[:, 0, :], in_=xb_v[:, 0, :])
        nc.sync.dma_start(out=wgp_sb, in_=wgp_d[:].rearrange(
            "p (h c) -> p h c", h=2))
        for g in range(1, n_grp):
            nc.sync.dma_start(out=xb_sb[:, g, :], in_=xb_v[:, g, :])

        # ---- pools ----
        th_pool = ctx.enter_context(tc.tile_pool(name="th", bufs=4))
        ch_pool = ctx.enter_context(tc.tile_pool(name="ch", bufs=1))
        hv_pool = ctx.enter_context(tc.tile_pool(name="hv", bufs=2))
        u_pool = ctx.enter_context(tc.tile_pool(name="u", bufs=4))
        t_pool = ctx.enter_context(tc.tile_pool(name="t", bufs=2))
        tt_pool = ctx.enter_context(tc.tile_pool(name="tt", bufs=2))
        out_pool = ctx.enter_context(tc.tile_pool(name="ob", bufs=3))

        # PSUM slots pad to one 2KB bank each; 8 banks total:
        # pjA 1 + tha 1 + mom 1 + ttp 1 + fin 2 + fin3 2
        pj_ps = ctx.enter_context(tc.tile_pool(name="pj_ps", bufs=1, space="PSUM"))
        tt_ps = ctx.enter_context(tc.tile_pool(name="tt_ps", bufs=1, space="PSUM"))
        fin_ps = ctx.enter_context(tc.tile_pool(name="fin_ps", bufs=2, space="PSUM"))
        fin3_ps = ctx.enter_context(tc.tile_pool(name="fin3_ps", bufs=2, space="PSUM"))

        mom = tt_ps.tile([128, n_grp, 2 * NK], F32, tag="mom", name="mom")
        out_v = out_d[:].rearrange("(g p) c -> p g c", p=128)

        # PE p-state warm-up: burn the 3us ramp before the first projection
        warm = fin_ps.tile([128, 512], F32, tag="fin", name="warm")
        for _ in range(26):
            nc.tensor.matmul(warm[:, 0:128], lhsT=ones_col, rhs=ones_col,
                             start=True, stop=True)

        th_tiles = {}
        msb_tiles = {}

        def stage_proj(g):
            gs = slice(128 * g, 128 * (g + 1))
            # phi^T then f^T ([i-part, 2h, s]), theta ([s-part, i]) last
            pja = pj_ps.tile([128, 2, 2, 128], F32, tag="pjA", name="pja")
            for i, (wsb, dst, bcol) in enumerate(((wpp_sb, ph_w, 136),
                                                  (wfp_sb, ff_w, 138))):
                acc = pja[:, i, :, :]
                for h in range(2):
                    for ch in range(n_ch):
                        nc.tensor.matmul(acc[:, h, :],
                                         lhsT=wsb[:, ch, :, 128*h:128*h+128],
                                         rhs=xtp_sb[:, ch, :, gs],
                                         start=(ch == 0), stop=(ch == n_ch - 1),
                                         perf_mode=DR)
                for h in range(2):
                    nc.scalar.activation(dst[:, h, gs], acc[:, h, :],
                                         mybir.ActivationFunctionType.Identity,
                                         bias=aux_sb[:, bcol + h:bcol + h + 1])
            th_acc = pj_ps.tile([128, K], F32, tag="tha", name="tha")
            for ch in range(n_ch):
                nc.tensor.matmul(th_acc, lhsT=xtp_sb[:, ch, :, gs],
                                 rhs=wtp_sb[:, ch, :, :],
                                 start=(ch == 0), stop=False, perf_mode=DR)
            nc.tensor.matmul(th_acc, lhsT=onespair, rhs=btp,
                             start=False, stop=True, perf_mode=DR)
            th = th_pool.tile([128, K], BF16, tag="th", name="th")
            nc.scalar.copy(th, th_acc)
            th_tiles[g] = th

        chain_tiles = {}

        def stage_chains(tag, ss, eng=None):
            """P-ladder (phi powers): DVE for the early solo groups (DVE is
            idle then), Pool for the wide tail groups (feeds Pool's own
            Q-muls with no cross-engine backflow)."""
            eng = eng or nc.vector
            phs = ph_w[:, :, ss]
            n_s = ss.stop - ss.start
            chain_tiles[(tag, 'P', 1)] = phs
            Pk = phs
            for k in range(2, DEG + 1):
                pn = ch_pool.tile([128, 2, n_s], BF16, tag=f"P{tag}{k}", name="pn")
                eng.tensor_mul(pn, Pk, phs)
                chain_tiles[(tag, 'P', k)] = pn
                Pk = pn

        def stage_q(g, tag, off):
            """Q_k = P_k * f for group g on Pool, slicing the (possibly
            wider) P tiles of `tag` at sample offset `off`."""
            sl = slice(off, off + 128)
            gs = slice(128 * g, 128 * (g + 1))
            ffs = ff_w[:, :, gs]
            chain_tiles[(g, 'Q', 0)] = ffs
            for k in range(1, DEG + 1):
                qn = ch_pool.tile([128, 2, 128], BF16, tag=f"Q{g}{k}", name="qn")
                nc.gpsimd.tensor_mul(qn, chain_tiles[(tag, 'P', k)][:, :, sl], ffs)
                chain_tiles[(g, 'Q', k)] = qn

        def stage_mom(g, tag, off):
            """moment matmuls for group g: den moments from the P tiles of
            `tag` at offset `off`, num moments from group-g Q tiles."""
            sl = slice(off, off + 128)
            for k in range(1, DEG + 1):
                # k=0 is skipped: S_0 = 256 exactly, folded into the poly as
                # an immediate scalar
                cc = aux_sb[:, 128 + k:129 + k]
                Pk = chain_tiles[(tag, 'P', k)]
                for h in range(2):
                    nc.tensor.matmul(mom[:, g, NK + k:NK + k + 1],
                                     lhsT=Pk[:, h, sl], rhs=cc,
                                     start=(h == 0), stop=(h == 1))
            for k in range(DEG + 1):
                cc = aux_sb[:, 128 + k:129 + k]
                Qk = chain_tiles[(g, 'Q', k)]
                for h in range(2):
                    nc.tensor.matmul(mom[:, g, k:k + 1], lhsT=Qk[:, h, :],
                                     rhs=cc, start=(h == 0), stop=(h == 1))
            msb = th_pool.tile([128, 2 * NK], F32, tag="msb", name="msb")
            nc.scalar.copy(msb, mom[:, g, :])
            msb_tiles[g] = msb

        t_tiles = {}
        u_tiles = {}

        def stage_u(g):
            th = th_tiles[g]
            u = u_pool.tile([128, K], BF16, tag="u", name="u")
            nc.vector.tensor_mul(u, th, th)
            u_tiles[g] = u

        def stage_poly(g, last=False):
            """num/den via linear pairs (4x tensor_scalar) + Horner in
            u=theta^2.  num ladder on DVE; den ladder on Pool.  For the
            last-emitted group both ladders are on DVE and fuse into wide
            [128, 2, 256] ops (u broadcast via a 0-stride pair dim)."""
            th = th_tiles.pop(g)
            msb = msb_tiles.pop(g)
            cm = lambda k: msb[:, k:k + 1]
            cs = lambda k: msb[:, NK + k:NK + k + 1]
            u = u_tiles.pop(g)
            if last:
                nd = []
                for m in range(4):
                    ndm = hv_pool.tile([128, 2, K], BF16, tag=f"N{m}", name="ndm")
                    nc.vector.tensor_scalar(ndm[:, 0, :], th, cm(2 * m + 1),
                                            cm(2 * m), AOP.mult, AOP.add)
                    s2 = float(COEFS[0] * K) if m == 0 else cs(2 * m)
                    nc.vector.tensor_scalar(ndm[:, 1, :], th, cs(2 * m + 1),
                                            s2, AOP.mult, AOP.add)
                    nd.append(ndm)
                urep = bass.AP(tensor=u.tensor, offset=u.offset,
                               ap=[u.ap[0], [0, 2]] + u.ap[1:])
                acc = nd[3]
                for m in (2, 1, 0):
                    a1 = hv_pool.tile([128, 2, K], BF16, tag="a1", name="a1")
                    nc.vector.tensor_mul(a1, acc, urep)
                    a2 = hv_pool.tile([128, 2, K], BF16, tag="a2", name="a2")
                    nc.vector.tensor_add(a2, a1, nd[m])
                    acc = a2
                num, den = acc[:, 0, :], acc[:, 1, :]
            else:
                L = []
                for m in range(4):
                    Lm = hv_pool.tile([128, K], BF16, tag=f"L{m}", name="Lm")
                    nc.vector.tensor_scalar(Lm, th, cm(2 * m + 1), cm(2 * m),
                                            AOP.mult, AOP.add)
                    L.append(Lm)
                D = []
                for m in range(4):
                    Dm = hv_pool.tile([128, K], BF16, tag=f"D{m}", name="Dm")
                    s2 = float(COEFS[0] * K) if m == 0 else cs(2 * m)
                    nc.vector.tensor_scalar(Dm, th, cs(2 * m + 1), s2,
                                            AOP.mult, AOP.add)
                    D.append(Dm)
                num = L[3]
                for m in (2, 1, 0):
                    nm1 = hv_pool.tile([128, K], BF16, tag="nm1", name="nm1")
                    nc.vector.tensor_mul(nm1, num, u)
                    nm2 = hv_pool.tile([128, K], BF16, tag="nm2", name="nm2")
                    nc.vector.tensor_add(nm2, nm1, L[m])
                    num = nm2
                den = D[3]
                for m in (2, 1, 0):
                    dm1 = hv_pool.tile([128, K], BF16, tag="dm1", name="dm1")
                    nc.gpsimd.tensor_mul(dm1, den, u)
                    dm2 = hv_pool.tile([128, K], BF16, tag="dm2", name="dm2")
                    nc.gpsimd.tensor_add(dm2, dm1, D[m])
                    den = dm2
            hinv = t_pool.tile([128, K], BF16, tag="hinv", name="hinv")
            with nc.allow_low_precision(reason="bf16 softmax denom reciprocal"):
                nc.vector.reciprocal(hinv, den)
            tb = t_pool.tile([128, K], BF16, tag="tb", name="tb")
            (nc.vector if last else nc.gpsimd).tensor_mul(tb, num, hinv)
            t_tiles[g] = tb

        def stage_tail(g, last=False, out_act=False):
            tb = t_tiles.pop(g)
            tp = tt_ps.tile([128, 2, 128], BF16, tag="ttp", name="ttp")
            for h in range(2):
                nc.tensor.transpose(tp[:, h, :], tb[:, 128 * h:128 * h + 128], ident)
            ttb = tt_pool.tile([128, 2, 128], F8, tag="tt", name="ttb")
            # the last group's ttb cast runs on DVE so it is not queued
            # behind the previous group's fin evacuation on ACT
            (nc.vector.tensor_copy if last else nc.scalar.copy)(ttb, tp)
            ob = out_pool.tile([128, c_dim], BF16, tag="ob", name="ob")
            n_chk = c_dim // 512
            for n in range(n_chk):
                csl = slice(512 * n, 512 * (n + 1))
                pool = fin3_ps if last else fin_ps
                fin = pool.tile([128, 512], F32, tag="fin3" if last else "fin",
                                name="fin")
                nc.tensor.matmul(fin, lhsT=onespair, rhs=bgp(csl),
                                 start=True, stop=False, perf_mode=DR)
                nc.tensor.matmul(fin, lhsT=ttb, rhs=wgp_sb[:, :, csl],
                                 start=False, stop=True, perf_mode=DR)
                if last:
                    # drain: DVE adds straight from PSUM (ACT is still busy
                    # evacuating the previous group); chunks stream out
                    nc.vector.tensor_add(ob[:, csl], fin, xb_sb[:, g, csl])
                    nc.sync.dma_start(out=out_v[:, g, csl], in_=ob[:, csl])
                else:
                    fsb = out_pool.tile([128, 512], BF16, tag=f"fs{n % 2}",
                                        name="fsb")
                    nc.scalar.copy(fsb, fin)
                    nc.gpsimd.tensor_add(ob[:, csl], fsb, xb_sb[:, g, csl])
                    if n % 2 == 1:
                        # each half leaves as soon as its two chunks land;
                        # the latest ACT-path group goes out via ACT so the
                        # fast-drain group's chunk DMAs get SP to themselves
                        hsl = slice(512 * (n - 1), 512 * (n + 1))
                        q = nc.scalar if out_act else nc.sync
                        q.dma_start(out=out_v[:, g, hsl], in_=ob[:, hsl])

        # ---- software-pipelined emission ----
        stage_proj(0)
        stage_proj(1)
        stage_chains("0", slice(0, 128))
        stage_u(0)
        stage_q(0, "0", 0)
        stage_proj(2)
        stage_chains("1", slice(128, 256))
        stage_u(1)
        stage_q(1, "1", 0)
        stage_mom(0, "0", 0)
        stage_proj(3)
        stage_poly(0)
        stage_mom(1, "1", 0)
        stage_chains("w", slice(256, 512))
        stage_u(2)
        stage_u(3)
        stage_q(2, "w", 0)
        stage_mom(2, "w", 0)
        stage_q(3, "w", 128)
        stage_mom(3, "w", 128)
        stage_poly(1)
        stage_tail(0)
        stage_poly(2)
        stage_tail(1)
        stage_tail(2, out_act=True)
        stage_poly(3, last=True)
        stage_tail(3, last=True)

_NC_CACHE = {}


def _get_nc(n_samp, c_dim):
    key = (n_samp, c_dim)
    if key not in _NC_CACHE:
        _NC_CACHE[key] = build_nc(n_samp, c_dim)
    return _NC_CACHE[key]


def _pack_dr(w, n_ch):
    """[C, K] -> DoubleRow pair layout [128, n_ch*2*K]:
    w[p, ch, h, i] = W[ch*256 + h*128 + p, i]."""
    Cd, Kd = w.shape
    return np.ascontiguousarray(
        w.reshape(n_ch, 2, 128, Kd).transpose(2, 0, 1, 3)).reshape(128, -1)


def _prep_shared(inputs, c_dim=C):
    n_ch = c_dim // 256
    f8 = lambda v: np.asarray(v, np.float32).astype(NPF8)
    aux = np.zeros((128, AUXW), np.float32)
    aux[:, :128] = np.eye(128, dtype=np.float32)
    aux[:, 128:128 + NK] = np.asarray(COEFS, np.float32)[None, :]
    bph = np.asarray(inputs["b_phi"], np.float32)
    bfv = np.asarray(inputs["b_f"], np.float32)
    for h in range(2):
        aux[:, 136 + h] = bph[128 * h:128 * h + 128]
        aux[:, 138 + h] = bfv[128 * h:128 * h + 128]
    bpk = np.zeros((1, BPW), np.float32)
    bpk[0, 0:K] = np.asarray(inputs["b_theta"], np.float32)
    bpk[0, K:K + c_dim] = np.asarray(inputs["b_g"], np.float32)
    wg = np.asarray(inputs["W_g"], np.float32)  # [K, c_dim]
    wgp = np.ascontiguousarray(
        wg.reshape(2, 128, c_dim).transpose(1, 0, 2)).reshape(128, -1)
    return {
        "wtp": _pack_dr(f8(inputs["W_theta"]), n_ch),
        "wpp": _pack_dr(f8(inputs["W_phi"]), n_ch),
        "wfp": _pack_dr(f8(inputs["W_f"]), n_ch),
        "wgp": np.ascontiguousarray(wgp.astype(NPF8)),
        "aux": aux.astype(NPBF),
        "bpk": bpk.astype(NPF8),
    }


def _build_in_maps(inputs):
    x = np.asarray(inputs["x"], dtype=np.float32)
    B, c_dim = x.shape
    n_samp = B // N_CORES
    n_ch = c_dim // 256
    shared = _prep_shared(inputs, c_dim)
    xb = np.ascontiguousarray(x.astype(NPBF))
    x8 = x.astype(NPF8)
    in_maps = []
    for c in range(N_CORES):
        xc = x8[c * n_samp:(c + 1) * n_samp]          # [n_samp, c_dim]
        xtp = np.ascontiguousarray(
            xc.T.reshape(n_ch, 2, 128, n_samp).transpose(2, 0, 1, 3)
        ).reshape(128, -1)
        m = {"xb": xb[c * n_samp:(c + 1) * n_samp], "xtp": xtp}
        m.update(shared)
        in_maps.append(m)
    return n_samp, c_dim, in_maps


def kernel(**inputs):
    n_samp, c_dim, in_maps = _build_in_maps(inputs)
    nc = _get_nc(n_samp, c_dim)
    res = run_bass_kernel_spmd(nc, in_maps, core_ids=list(range(N_CORES)))
    return np.concatenate([res.results[c]["out"] for c in range(N_CORES)],
                          axis=0).astype(np.float32)
